# revision 4
# baseline (speedup 1.0000x reference)
"""ColumnParallelLinearWithMoE Trainium2 kernel.

Expert-parallel: expert e -> NeuronCore e. Each core computes
    y_e [8192, 512] = x_e [8192, 1024] @ W_e.T [1024, 512] (+ b_e)
where x_e = input_[idx_list[e]] flattened over (per, seq).

Routing gather/scatter and the x transpose (to put the contraction dim on
SBUF partitions) happen on the host; the device does the dense matmul in
bf16 and stores bf16 (PSUM accumulation is fp32). The bias is zero in this
module (zero-initialized); if a nonzero bias ever shows up it is applied
on the host during unshard.

v2 layout notes (vs the 131us baseline):
- No on-device bias add: PSUM is evicted by a single DVE tensor_copy per
  4-bank PSUM tile (fp32 -> bf16), which also halves store traffic.
- 4-bank PSUM tiles ([128, 4, 512]) cut the PSUM/evict/store semaphore
  count ~4x. The TileContext teardown emits per-semaphore clear
  instructions (~115ns each, ~57 per engine in the baseline = 6.5us of
  pure epilogue), so fewer semaphores directly shortens the kernel.
- Ramp: x-chunk loads on the sync ring and w loads on the scalar ring
  issue in parallel from t=0; a short PE warmup bridges the ~3us until
  the first real matmul so HAM un-throttles early.
- The very last PSUM tile is evicted/stored per-j (128-token granularity)
  so the tail drain after the final matmul is ~1.4us.
"""

import sys

if "/opt/trn_rl_repo" not in sys.path:
    sys.path.insert(0, "/opt/trn_rl_repo")

import numpy as np

# Problem constants (hardcoded per harness contract).
E = 8
BS = 64
S = 1024
D = 1024
OPP = 512
P = 128
TOK = (BS // E) * S  # 8192 tokens per expert
KT = D // P          # 8 contraction tiles
TW = 1024            # token-superblock width staged in SBUF
NSUP = TOK // TW     # 8 superblocks
TPS = TW // P        # 8 token tiles (of 128) per superblock
NH = 2               # PSUM tiles per superblock (4 banks each)
JH = TPS // NH       # 4 token tiles per PSUM tile

N_WARM = 6           # warmup matmuls bridging the initial DMA window

_programs: dict[str, tuple] = {}


def _build():
    import concourse.bacc as bacc
    import concourse.tile as tile
    from concourse import mybir
    import ml_dtypes

    mm_dt = mybir.dt.bfloat16
    np_in = ml_dtypes.bfloat16

    nc = bacc.Bacc(None, target_bir_lowering=False, debug=False)

    xt = nc.dram_tensor("xt", [D, TOK], mm_dt, kind="ExternalInput")
    wt = nc.dram_tensor("wt", [D, OPP], mm_dt, kind="ExternalInput")
    y = nc.dram_tensor("y", [TOK, OPP], mm_dt, kind="ExternalOutput")

    xt_r = xt.rearrange("(k p) t -> p k t", p=P)                 # [128, KT, TOK]
    wt_r = wt.rearrange("(k p) c -> p k c", p=P)                 # [128, KT, OPP]
    y_r = y.rearrange("(s h j p) c -> p s h j c", p=P, j=JH, h=NH)  # [128,NSUP,NH,JH,OPP]

    with tile.TileContext(nc) as tc:
        with (
            tc.tile_pool(name="wpool", bufs=1) as wpool,
            tc.tile_pool(name="xpool", bufs=4) as xpool,
            tc.tile_pool(name="opool", bufs=3) as opool,
            tc.tile_pool(name="pspool", bufs=2, space="PSUM") as pspool,
        ):
            # Ramp: x token-chunks on the sync ring and w on the scalar ring
            # issue in parallel so the first accumulation group's operands
            # (x chunk 0, w[k=0]) land ~2us in. Chunk sizes double so the
            # issue rate (~0.6us per dma_start) keeps ahead of the consume
            # rate (one 128-token tile per ~1.7us warm).
            x0_sb = xpool.tile([P, KT, TW], mm_dt, tag="x")
            w_sb = wpool.tile([P, KT, OPP], mm_dt)
            nc.sync.dma_start(out=x0_sb[:, :, 0:128], in_=xt_r[:, :, 0:128])
            nc.scalar.dma_start(out=w_sb[:, 0:2, :], in_=wt_r[:, 0:2, :])
            nc.sync.dma_start(out=x0_sb[:, :, 128:256], in_=xt_r[:, :, 128:256])
            nc.scalar.dma_start(out=w_sb[:, 2:, :], in_=wt_r[:, 2:, :])
            nc.sync.dma_start(out=x0_sb[:, :, 256:512], in_=xt_r[:, :, 256:512])
            nc.sync.dma_start(out=x0_sb[:, :, 512:1024], in_=xt_r[:, :, 512:1024])

            # PE warmup on a zeroed tile: keeps the PE busy from ~0.3us so
            # HAM un-throttles (1.2 -> 2.4 GHz) around the time the first
            # real matmul's operands arrive. Results are never read.
            warm_src = wpool.tile([P, OPP], mm_dt, tag="warm")
            nc.gpsimd.memset(warm_src[:], 0.0)
            # Same shape/tag as the real PSUM tiles so the pool stays at
            # bufs=2 x 4 banks; warm matmuls precede all real ones in the
            # Tensor FIFO so the buf-0 reuse needs no cross-engine sync.
            warm_ps = pspool.tile([P, JH, OPP], mybir.dt.float32, tag="ps")
            for _ in range(N_WARM):
                nc.tensor.matmul(
                    warm_ps[:, 0, :], warm_src[:, :P], warm_src[:],
                    start=True, stop=True,
                )

            for s in range(NSUP):
                if s == 0:
                    x_sb = x0_sb
                else:
                    x_sb = xpool.tile([P, KT, TW], mm_dt, tag="x")
                    nc.sync.dma_start(
                        out=x_sb[:], in_=xt_r[:, :, s * TW : (s + 1) * TW]
                    )
                for h in range(NH):
                    ps = pspool.tile([P, JH, OPP], mybir.dt.float32, tag="ps")
                    for jj in range(JH):
                        j = h * JH + jj
                        for k in range(KT):
                            nc.tensor.matmul(
                                ps[:, jj, :],
                                x_sb[:, k, j * P : (j + 1) * P],
                                w_sb[:, k, :],
                                start=(k == 0),
                                stop=(k == KT - 1),
                            )
                    o_sb = opool.tile([P, JH, OPP], mm_dt, tag="o")
                    last = (s == NSUP - 1) and (h == NH - 1)
                    if not last:
                        nc.vector.tensor_copy(o_sb[:], ps[:])
                        nc.scalar.dma_start(out=y_r[:, s, h, :, :], in_=o_sb[:])
                    else:
                        # Fine-grained tail: evict/store each 128-token tile
                        # as soon as its accumulation group stops.
                        for jj in range(JH):
                            nc.vector.tensor_copy(o_sb[:, jj, :], ps[:, jj, :])
                            nc.scalar.dma_start(
                                out=y_r[:, s, h, jj, :], in_=o_sb[:, jj, :]
                            )

    nc.compile()
    return nc, np_in


def _get_program():
    if "v2" not in _programs:
        _programs["v2"] = _build()
    return _programs["v2"]


def kernel(input_, idx_list, W, b, **_ignored):
    from concourse.bass_utils import run_bass_kernel_spmd

    input_ = np.asarray(input_)
    idx = np.asarray(idx_list).astype(np.int64)
    W = np.asarray(W, dtype=np.float32)
    b = np.asarray(b, dtype=np.float32)

    nc, np_in = _get_program()

    in_maps = []
    for e in range(E):
        xg = input_[idx[e]].reshape(TOK, D).astype(np.float32, copy=False)
        xtr = np.ascontiguousarray(xg.T).astype(np_in)
        wtr = np.ascontiguousarray(W[e].T).astype(np_in)
        in_maps.append({"xt": xtr, "wt": wtr})

    res = run_bass_kernel_spmd(nc, in_maps, core_ids=list(range(E)))

    out = np.zeros((BS, S, E * OPP), dtype=input_.dtype)
    for e in range(E):
        ye = np.asarray(res.results[e]["y"]).astype(input_.dtype)
        ye = ye.reshape(BS // E, S, OPP)
        if b[e].any():
            ye = ye + b[e][None, None, :]
        out[idx[e], :, e * OPP : (e + 1) * OPP] = ye
    return out


# revision 5
# speedup vs baseline: 1.0400x; 1.0400x over previous
"""ColumnParallelLinearWithMoE Trainium2 kernel.

Expert-parallel: expert e -> NeuronCore e. Each core computes
    y_e [8192, 512] = x_e [8192, 1024] @ W_e.T [1024, 512] (+ b_e)
where x_e = input_[idx_list[e]] flattened over (per, seq).

Routing gather/scatter and the x transpose (to put the contraction dim on
SBUF partitions) happen on the host; the device does the dense matmul in
bf16 (fp32 PSUM accumulation) and stores bf16. The bias is zero in this
module (zero-initialized); if a nonzero bias ever shows up it is applied
on the host during unshard, so the device pipeline is matmul -> DVE copy
(fp32->bf16 downcast) -> store, with no bias stage.

Timing structure (measured):
- The exec clock starts at the first user instruction and ends after a
  fixed ~7us TileContext teardown (254 semaphore clears spread over the
  5 engines) that does not depend on kernel structure.
- Body floor is 512 matmuls x 216ns = 110.6us (N=512 moving operand,
  warm PE). The job of everything else is to start the matmul stream
  early and to finish draining quickly after the last matmul.
- Ramp: x token-chunks and w k-chunks issue immediately; a PE warmup on
  a zeroed tile bridges the DMA window so HAM un-throttles (1.2->2.4GHz)
  by the time real matmuls start. HWDGE issue costs ~0.6-0.8us per
  dma_start on the issuing engine and scales with transfer size, so the
  ramp uses many small transfers while steady-state uses one per super.
- PSUM dependency tracking is tile-granular: per-j [128,512] PSUM tiles
  are required so each eviction only waits on its own 8-matmul group.
- The last super evicts/stores per token-tile so the post-matmul drain
  is ~1.6us.
"""

import sys

if "/opt/trn_rl_repo" not in sys.path:
    sys.path.insert(0, "/opt/trn_rl_repo")

import numpy as np

# Problem constants (hardcoded per harness contract).
E = 8
BS = 64
S = 1024
D = 1024
OPP = 512
P = 128
TOK = (BS // E) * S  # 8192 tokens per expert
KT = D // P          # 8 contraction tiles
TW = 1024            # token-superblock width staged in SBUF
NSUP = TOK // TW     # 8 superblocks
TPS = TW // P        # 8 token tiles (of 128) per superblock

N_WARM = 16          # warmup matmuls bridging the initial DMA window

_programs: dict[str, tuple] = {}


def _build():
    import concourse.bacc as bacc
    import concourse.tile as tile
    from concourse import mybir
    import ml_dtypes

    mm_dt = mybir.dt.bfloat16
    np_in = ml_dtypes.bfloat16

    nc = bacc.Bacc(None, target_bir_lowering=False, debug=False)

    xt = nc.dram_tensor("xt", [D, TOK], mm_dt, kind="ExternalInput")
    wt = nc.dram_tensor("wt", [D, OPP], mm_dt, kind="ExternalInput")
    y = nc.dram_tensor("y", [TOK, OPP], mm_dt, kind="ExternalOutput")

    xt_r = xt.rearrange("(k p) t -> p k t", p=P)           # [128, KT, TOK]
    wt_r = wt.rearrange("(k p) c -> p k c", p=P)           # [128, KT, OPP]
    y_r = y.rearrange("(s j p) c -> p s j c", p=P, j=TPS)  # [128, NSUP, TPS, OPP]

    with tile.TileContext(nc) as tc:
        with (
            tc.tile_pool(name="wpool", bufs=1) as wpool,
            tc.tile_pool(name="xpool", bufs=4) as xpool,
            tc.tile_pool(name="opool", bufs=2) as opool,
            tc.tile_pool(name="pspool", bufs=8, space="PSUM") as pspool,
        ):
            # PE prewarm: dummy matmuls on a zeroed tile bridging the
            # initial-load window so HAM un-throttles (1.2 -> 2.4 GHz) and
            # stays warm until the first real matmul. Results never read.
            warm_src = wpool.tile([P, OPP], mm_dt, tag="warm")
            nc.gpsimd.memset(warm_src[:], 0.0)
            warm_ps = pspool.tile([P, OPP], mybir.dt.float32, tag="ps")
            for _ in range(N_WARM):
                nc.tensor.matmul(
                    warm_ps[:], warm_src[:, :P], warm_src[:], start=True, stop=True
                )

            # Ramp: the first token-tile's x chunk and w[k=0] land first so
            # the first accumulation group starts early; everything else
            # arrives while the PE chews through the warmup.
            x0_sb = xpool.tile([P, KT, TW], mm_dt, tag="x")
            w_sb = wpool.tile([P, KT, OPP], mm_dt)
            nc.sync.dma_start(out=x0_sb[:, :, 0:P], in_=xt_r[:, :, 0:P])
            nc.sync.dma_start(out=w_sb[:, 0, :], in_=wt_r[:, 0, :])
            nc.sync.dma_start(out=w_sb[:, 1:4, :], in_=wt_r[:, 1:4, :])
            nc.sync.dma_start(out=w_sb[:, 4:, :], in_=wt_r[:, 4:, :])
            for j in range(1, TPS):
                nc.sync.dma_start(
                    out=x0_sb[:, :, j * P : (j + 1) * P],
                    in_=xt_r[:, :, j * P : (j + 1) * P],
                )

            for s in range(NSUP):
                if s == 0:
                    x_sb = x0_sb
                elif s == 1:
                    # Super 1 races the tail of the preload; half-chunks keep
                    # each PE gap under the 3.4us HAM re-throttle window.
                    x_sb = xpool.tile([P, KT, TW], mm_dt, tag="x")
                    H = TW // 2
                    for c in range(2):
                        nc.sync.dma_start(
                            out=x_sb[:, :, c * H : (c + 1) * H],
                            in_=xt_r[:, :, TW + c * H : TW + (c + 1) * H],
                        )
                else:
                    x_sb = xpool.tile([P, KT, TW], mm_dt, tag="x")
                    nc.sync.dma_start(
                        out=x_sb[:], in_=xt_r[:, :, s * TW : (s + 1) * TW]
                    )
                o_sb = opool.tile([P, TPS, OPP], mm_dt, tag="o")
                last_s = s == NSUP - 1
                for j in range(TPS):
                    ps = pspool.tile([P, OPP], mybir.dt.float32, tag="ps")
                    for k in range(KT):
                        nc.tensor.matmul(
                            ps[:],
                            x_sb[:, k, j * P : (j + 1) * P],
                            w_sb[:, k, :],
                            start=(k == 0),
                            stop=(k == KT - 1),
                        )
                    nc.vector.tensor_copy(o_sb[:, j, :], ps[:])
                    if last_s:
                        # Fine-grained tail: store each token tile as soon
                        # as it is evicted.
                        nc.scalar.dma_start(
                            out=y_r[:, s, j, :], in_=o_sb[:, j, :]
                        )
                if not last_s:
                    nc.scalar.dma_start(out=y_r[:, s, :, :], in_=o_sb[:])

    nc.compile()
    return nc, np_in


def _get_program():
    if "v3" not in _programs:
        _programs["v3"] = _build()
    return _programs["v3"]


def kernel(input_, idx_list, W, b, **_ignored):
    from concourse.bass_utils import run_bass_kernel_spmd

    input_ = np.asarray(input_)
    idx = np.asarray(idx_list).astype(np.int64)
    W = np.asarray(W, dtype=np.float32)
    b = np.asarray(b, dtype=np.float32)

    nc, np_in = _get_program()

    in_maps = []
    for e in range(E):
        xg = input_[idx[e]].reshape(TOK, D).astype(np.float32, copy=False)
        xtr = np.ascontiguousarray(xg.T).astype(np_in)
        wtr = np.ascontiguousarray(W[e].T).astype(np_in)
        in_maps.append({"xt": xtr, "wt": wtr})

    res = run_bass_kernel_spmd(nc, in_maps, core_ids=list(range(E)))

    out = np.zeros((BS, S, E * OPP), dtype=input_.dtype)
    for e in range(E):
        ye = np.asarray(res.results[e]["y"]).astype(input_.dtype)
        ye = ye.reshape(BS // E, S, OPP)
        if b[e].any():
            ye = ye + b[e][None, None, :]
        out[idx[e], :, e * OPP : (e + 1) * OPP] = ye
    return out
